# revision 30
# baseline (speedup 1.0000x reference)
"""DWT-based Perona-Malik diffusion block on 8 Trainium2 NeuronCores.

Math (see reference): one level of orthonormal Haar DWT, PM diffusion of the
detail subbands computed from batch 0 only and broadcast to every batch, IDWT,
then conv3x3 -> BN -> relu -> conv3x3 -> BN -> +feat.

Algebraic reductions:
  g = 1 / (1 + (e^2 + f^2)/2) with e = a - d, f = b - c of batch 0's 2x2
  blocks; u = g*e/2, v = g*f/2 (computed on host, shipped as bf16).
  Per batch, with s1 = a + d, s2 = b + c:
    feat[2i,2j]   = s1/2 + u     feat[2i,2j+1] = s2/2 + v
    feat[2i+1,2j] = s2/2 - v     feat[2i+1,2j+1] = s1/2 - u
  BN folds into the conv weights/biases (host).

Device layout: partition = ch + 64*row_parity (loaded that way straight from
DRAM); columns stored parity-split ([2, 130] sections with guard cols) so all
DVE writes are packed bf16 (4x mode). The 3x3 convs run as bf16 matmuls over
stacked (channel x row-parity) partitions: per output 2-pair psum bank, 6
accumulation passes -- 3 "A" taps (same row-pair, dense 128x128 weights) from
the natural tile, 3 "BC" taps (adjacent row pairs) from a partition-swapped,
slot-shifted companion tile (frBC/ztBC) with block-diagonal weights.

Sharding: pure data parallelism, 2 batches per core; u,v replicated.
"""

import sys

for _p in ("/opt/pypackages", "/opt/trn_rl_repo"):
    if _p not in sys.path:
        sys.path.insert(0, _p)

import numpy as np
import ml_dtypes

import concourse.bass as bass
import concourse.mybir as mybir
import concourse.tile as tile_mod
from concourse.bass_utils import run_bass_kernel_spmd
from concourse.tile import TileContext
from concourse.vector_clock import ScopedClock

F32 = mybir.dt.float32
BF16 = mybir.dt.bfloat16
AF = mybir.ActivationFunctionType
ALU = mybir.AluOpType
BFNP = ml_dtypes.bfloat16

N_CORES = 8
B, C, H, W = 16, 64, 256, 256
BPC = B // N_CORES  # batches per core
NPAIR = H // 2  # 128 row pairs per image
G = 8  # row pairs per chunk
NCHUNK = NPAIR // G  # 16
NQ = W // 2  # column pairs
SEC = NQ + 2  # per-parity-section cols incl guard cols 0 and NQ+1
EPS = 1e-5


def _patched_drain_and_barrier(self, tick_clock, wait_clock):
    # This walrus build rejects >1 sync-wait command per instruction; put the
    # tile-exit drain's waits on individual nops instead.
    nc = self.nc
    collector = nc.sync.nop(nofuse=True)
    wait_clock.add_sem_waits(
        collector.ins, ScopedClock({None: tick_clock.global_clock})
    )
    si = collector.ins.sync_info
    waits = list(si.on_wait) if si is not None else []
    if si is not None:
        si.on_wait = waits[:1]
    for w in waits[1:]:
        n = nc.sync.nop(nofuse=True)
        n.ins.sync_info = mybir.SyncInfo(on_wait=[w], on_update=[])
    nc.sync.drain()
    nc.all_engine_barrier()
    popped = nc._tile_sem_poison_stack.pop()
    assert popped is self._sem_poison
    nc.clear_and_free_semaphores(list(self.sems.allocated().values()))
    nc.all_engine_barrier()


tile_mod.TileContext._drain_and_barrier = _patched_drain_and_barrier

# NOTE: unlike the f32r predecessor, this kernel's bf16 weights legalize into
# standalone InstLdweights, which walrus rejects under --enable-ldw-opt=true;
# the default (=false) is required, so no run_command patching here.

F32R = mybir.dt.float32r


def split_multi_waits(nc):
    """Move extra sync-waits onto preceding single-wait nops (same engine)."""
    for fn in nc.m.functions:
        for blk in fn.blocks:
            new_insts = []
            for inst in blk.instructions:
                si = inst.sync_info
                waits = list(si.on_wait) if si is not None else []
                if len(waits) > 1:
                    for w in waits[:-1]:
                        n = mybir.InstNoOp(
                            name=nc.get_next_instruction_name(), ins=[], outs=[]
                        )
                        n.engine = inst.engine
                        n.bass_nofuse = True
                        n.sync_info = mybir.SyncInfo(on_wait=[w], on_update=[])
                        new_insts.append(n)
                    si.on_wait = waits[-1:]
                new_insts.append(inst)
            blk.instructions = new_insts


# Tap tables: per output column parity, the 6 accumulation passes of a conv.
# Entries: (w_col_offset, src_kind, src_parity, col_shift); w cols 0..2 are
# the dense same-pair "A" taps (kx 0..2), 3..5 the merged adjacent-pair "BC"
# taps. src_kind 0 = natural tile (fr/zt), 1 = companion tile (frBC/ztBC).
TAPS = {
    0: [(0, 0, 1, -1), (1, 0, 0, 0), (2, 0, 1, 0),
        (3, 1, 1, -1), (4, 1, 0, 0), (5, 1, 1, 0)],
    1: [(0, 0, 0, 0), (1, 0, 1, 0), (2, 0, 0, 1),
        (3, 1, 0, 0), (4, 1, 1, 0), (5, 1, 0, 1)],
}


def _build_nc():
    nc = bass.Bass("TRN2", target_bir_lowering=False, debug=False,
                   num_devices=N_CORES)

    xs_d = nc.dram_tensor("xs", [BPC, C, H, W], BF16, kind="ExternalInput").ap()
    uv_d = nc.dram_tensor("uv", [128, NPAIR, NQ], BF16,
                          kind="ExternalInput").ap()
    wts_d = nc.dram_tensor("wts", [128, 12, 128], F32R,
                           kind="ExternalInput").ap()
    b1_d = nc.dram_tensor("bias1", [128, 1], F32, kind="ExternalInput").ap()
    b2_d = nc.dram_tensor("bias2", [128, 1], F32, kind="ExternalInput").ap()
    out_d = nc.dram_tensor("out", [BPC, C, H, W], BF16,
                           kind="ExternalOutput").ap()

    NIT = BPC * NCHUNK  # 32 flat chunk iterations

    with TileContext(nc) as tc, nc.allow_low_precision(
        reason="2e-2 rel-err budget tolerates bf16 activations and weights"
    ):
        with (
            tc.tile_pool(name="const", bufs=1) as cpool,
            tc.tile_pool(name="xin", bufs=2) as xpool,
            tc.tile_pool(name="s12", bufs=2) as spool,
            tc.tile_pool(name="fr", bufs=3) as frpool,
            tc.tile_pool(name="frbc", bufs=2) as fbpool,
            tc.tile_pool(name="zt", bufs=3) as zpool,
            tc.tile_pool(name="ztbc", bufs=3) as zbpool,
            tc.tile_pool(name="outb", bufs=2) as opool,
            tc.tile_pool(name="psum1", bufs=2, space="PSUM") as p1pool,
            tc.tile_pool(name="psum2", bufs=2, space="PSUM") as p2pool,
        ):
            wts_sb = cpool.tile([128, 12, 128], F32R)
            b1_sb = cpool.tile([128, 1], F32)
            b2_sb = cpool.tile([128, 1], F32)
            uv_sb = cpool.tile([128, NPAIR, NQ], BF16)

            def load_consts():
                # first x chunk is already in flight; uv streams in pieces
                # behind the early x loads so prep never waits on the bulk
                nc.sync.dma_start(
                    out=uv_sb[:, 0 : 2 * G, :], in_=uv_d[:, 0 : 2 * G, :]
                )
                nc.sync.dma_start(out=wts_sb[:], in_=wts_d[:])
                nc.sync.dma_start(out=b1_sb[:], in_=b1_d[:])
                nc.sync.dma_start(out=b2_sb[:], in_=b2_d[:])

            def load_uv_piece(j):
                # pieces of 28 pairs at iterations -1..2 (16 + 4*28 = 128)
                lo = 2 * G + 28 * j
                hi = min(lo + 28, NPAIR)
                if lo < hi:
                    nc.sync.dma_start(
                        out=uv_sb[:, lo:hi, :], in_=uv_d[:, lo:hi, :]
                    )

            x_tiles = {}
            fr_tiles = {}
            frbc_tiles = {}
            z_tiles = {}
            zbc_tiles = {}

            def load(i):
                bi, k = divmod(i, NCHUNK)
                # partition = channel only; row parity lives in the free dim
                # so the cross-parity s1/s2 adds have equal input bases
                xc = xpool.tile([64, G, 2, W], BF16)
                x_tiles[i] = xc
                rows = xs_d[bi, :, 2 * G * k : 2 * G * (k + 1), :].rearrange(
                    "c (j r) w -> c j r w", r=2
                )
                if i == 0:
                    # pipeline fill: first 5 pairs arrive (and flow) first
                    nc.sync.dma_start(out=xc[:, 0:5], in_=rows[:, 0:5])
                    nc.sync.dma_start(out=xc[:, 5:G], in_=rows[:, 5:G])
                else:
                    nc.sync.dma_start(out=xc[:], in_=rows)

            def prep(i):
                bi, k = divmod(i, NCHUNK)
                xc = x_tiles.pop(i)
                s12 = spool.tile([128, G, NQ], BF16)
                fr = frpool.tile([128, G, 2, SEC], F32R)
                fr_tiles[i] = fr
                body = slice(1, 1 + NQ)
                subs = (slice(0, 5), slice(5, G)) if i == 0 else (slice(0, G),)
                for sl in subs:
                    pr = slice(G * k + sl.start, G * k + sl.stop)
                    u_ap = uv_sb[0:64, pr, :]
                    v_ap = uv_sb[64:128, pr, :]
                    nc.vector.tensor_add(
                        out=s12[0:64, sl], in0=xc[:, sl, 0, 0::2],
                        in1=xc[:, sl, 1, 1::2],
                    )
                    nc.vector.tensor_add(
                        out=s12[64:128, sl], in0=xc[:, sl, 0, 1::2],
                        in1=xc[:, sl, 1, 0::2],
                    )
                    # feat2 = 2*feat = s +/- u2 (the 1/2 is folded into w1 and
                    # the host-side output halving)
                    nc.vector.tensor_add(
                        out=fr[0:64, sl, 0, body], in0=s12[0:64, sl], in1=u_ap
                    )
                    nc.vector.tensor_add(
                        out=fr[0:64, sl, 1, body], in0=s12[64:128, sl],
                        in1=v_ap,
                    )
                    nc.vector.tensor_sub(
                        out=fr[64:128, sl, 0, body], in0=s12[64:128, sl],
                        in1=v_ap,
                    )
                    nc.vector.tensor_sub(
                        out=fr[64:128, sl, 1, body], in0=s12[0:64, sl],
                        in1=u_ap,
                    )
                nc.vector.memset(fr[:, :, :, 0:1].bitcast(F32), 0.0)
                nc.vector.memset(fr[:, :, :, SEC - 1 : SEC].bitcast(F32), 0.0)

            def build_bc_early(i):
                """frBC main body: needs only fr(i) and fr(i-1) -- emitted
                right after prep(i) so the copies land well before conv1(i)."""
                bi, k = divmod(i, NCHUNK)
                dst = fbpool.tile([128, G, 2, SEC], F32R)
                frbc_tiles[i] = dst
                src = fr_tiles[i]
                if i == 0:
                    nc.gpsimd.dma_start(out=dst[0:64, 1:5], in_=src[64:128, 0:4])
                    nc.gpsimd.dma_start(out=dst[64:128, 0:4], in_=src[0:64, 1:5])
                    nc.gpsimd.dma_start(
                        out=dst[0:64, 5:G], in_=src[64:128, 4 : G - 1]
                    )
                    nc.gpsimd.dma_start(
                        out=dst[64:128, 4 : G - 1], in_=src[0:64, 5:G]
                    )
                else:
                    nc.gpsimd.dma_start(
                        out=dst[0:64, 1:G], in_=src[64:128, 0 : G - 1]
                    )
                    nc.gpsimd.dma_start(
                        out=dst[64:128, 0 : G - 1], in_=src[0:64, 1:G]
                    )
                if k == 0:
                    nc.vector.memset(dst[0:64, 0:1].bitcast(F32), 0.0)
                else:
                    nc.gpsimd.dma_start(
                        out=dst[0:64, 0:1],
                        in_=fr_tiles[i - 1][64:128, G - 1 : G],
                    )

            def build_bc_late(i):
                """frBC forward halo: needs fr(i+1)."""
                bi, k = divmod(i, NCHUNK)
                dst = frbc_tiles[i]
                if k == NCHUNK - 1:
                    nc.vector.memset(dst[64:128, G - 1 : G].bitcast(F32), 0.0)
                else:
                    nc.gpsimd.dma_start(
                        out=dst[64:128, G - 1 : G],
                        in_=fr_tiles[i + 1][0:64, 0:1],
                    )

            def emit_conv(psum, wbase, half, nat, comp):
                """12 matmuls (512 rows each): one half-chunk (4 pairs) of one
                conv. Each output col-parity accumulates in its own psum bank;
                weight-major so each w col is loaded once, used twice."""
                ts = slice(4 * half, 4 * half + 4)
                for wcol in range(6):
                    w_ap = wts_sb[:, wbase + wcol, :]
                    for p_out in (0, 1):
                        _, kind, p_in, shift = TAPS[p_out][wcol]
                        src = (nat if kind == 0 else comp)[
                            :, ts, p_in, 1 + shift : 1 + shift + NQ
                        ]
                        nc.tensor.matmul(
                            psum[:, p_out, :, :],
                            w_ap,
                            src,
                            start=(wcol == 0),
                            stop=(wcol == 5),
                        )

            def zbc_alloc(j):
                bi, k = divmod(j, NCHUNK)
                zbc = zbpool.tile([128, G, 2, SEC], F32R)
                zbc_tiles[j] = zbc
                nc.vector.memset(zbc[:, :, :, 0:1].bitcast(F32), 0.0)
                nc.vector.memset(zbc[:, :, :, SEC - 1 : SEC].bitcast(F32), 0.0)
                if k == 0:
                    nc.vector.memset(zbc[0:64, 0:1].bitcast(F32), 0.0)
                if k == NCHUNK - 1:
                    nc.vector.memset(zbc[64:128, G - 1 : G].bitcast(F32), 0.0)

            def conv1(i, halves):
                bi, k = divmod(i, NCHUNK)
                fr = fr_tiles[i]
                fbc = frbc_tiles[i]
                if 0 in halves:
                    zt = zpool.tile([128, G, 2, SEC], F32R)
                    z_tiles[i] = zt
                else:
                    zt = z_tiles[i]
                zbc = zbc_tiles[i]
                body = slice(1, 1 + NQ)

                def act(out_ap, in_ap, bias):
                    nc.scalar.activation(
                        out_ap, in_ap, AF.Relu, bias=bias, scale=1.0
                    )

                bl = b1_sb[0:64, 0:1]
                bh = b1_sb[64:128, 0:1]
                for h in halves:
                    ps = p1pool.tile([128, 2, 4, NQ], F32)
                    emit_conv(ps, 0, h, fr, fbc)
                    for p in (0, 1):
                        # natural tile: both parities
                        act(zt[:, 4 * h : 4 * h + 4, p, body],
                            ps[:, p, :, :], b1_sb[:, 0:1])
                        # companion: par1 -> zbc[0:64] at slot pair+1,
                        # par0 -> zbc[64:128] at slot pair-1
                        if h == 0:
                            act(zbc[0:64, 1:5, p, body],
                                ps[64:128, p, 0:4, :], bh)
                            if k > 0:
                                act(zbc_tiles[i - 1][64:128, G - 1 : G, p, body],
                                    ps[0:64, p, 0:1, :], bl)
                            act(zbc[64:128, 0:3, p, body],
                                ps[0:64, p, 1:4, :], bl)
                        else:
                            act(zbc[0:64, 5:8, p, body],
                                ps[64:128, p, 0:3, :], bh)
                            if k < NCHUNK - 1:
                                act(zbc_tiles[i + 1][0:64, 0:1, p, body],
                                    ps[64:128, p, 3:4, :], bh)
                            act(zbc[64:128, 3:7, p, body],
                                ps[0:64, p, 0:4, :], bl)
                if 0 in halves:
                    nc.vector.memset(zt[:, :, :, 0:1].bitcast(F32), 0.0)
                    nc.vector.memset(
                        zt[:, :, :, SEC - 1 : SEC].bitcast(F32), 0.0
                    )

            def conv2(i):
                bi, k = divmod(i, NCHUNK)
                zt = z_tiles[i]
                zbc = zbc_tiles[i]
                fr = fr_tiles[i]
                ot = opool.tile([128, G, W], BF16)
                for h in range(2):
                    ps = p2pool.tile([128, 2, 4, NQ], F32)
                    emit_conv(ps, 6, h, zt, zbc)
                    for p in (0, 1):
                        nc.vector.scalar_tensor_tensor(
                            out=ot[:, 4 * h : 4 * h + 4, p::2],
                            in0=ps[:, p, :, :],
                            scalar=b2_sb[:, 0:1],
                            in1=fr[:, 4 * h : 4 * h + 4, p, 1 : 1 + NQ].bitcast(F32),
                            op0=ALU.add, op1=ALU.add,
                        )
                orows = out_d[bi, :, 2 * G * k : 2 * G * (k + 1), :].rearrange(
                    "c (j r) w -> c j r w", r=2
                )
                if i == NIT - 1:
                    # tail latency: ship each 4-pair half as soon as ready
                    for hh in range(2):
                        for r in range(2):
                            nc.sync.dma_start(
                                out=orows[:, 4 * hh : 4 * hh + 4, r, :],
                                in_=ot[64 * r : 64 * (r + 1),
                                       4 * hh : 4 * hh + 4],
                            )
                else:
                    for r in range(2):
                        nc.sync.dma_start(
                            out=orows[:, :, r, :], in_=ot[64 * r : 64 * (r + 1)]
                        )


            for i in range(-1, NIT + 1):
                if 0 <= i + 1 < NIT:
                    load(i + 1)
                    if i == -1:
                        load_consts()
                    if -1 <= i <= 2:
                        load_uv_piece(i + 1)
                    zbc_alloc(i + 1)
                    prep(i + 1)
                if 0 <= i < NIT:
                    build_bc_early(i)
                    build_bc_late(i)
                    conv1(i, (0,))
                if 0 <= i - 1 < NIT:
                    conv2(i - 1)
                if 0 <= i < NIT:
                    conv1(i, (1,))

    split_multi_waits(nc)
    return nc


_NC_CACHE = {}


def _get_nc():
    if "nc" not in _NC_CACHE:
        _NC_CACHE["nc"] = _build_nc()
    return _NC_CACHE["nc"]


def _host_prep(x0, w1, b1, g1, be1, m1, v1, w2, b2, g2, be2, m2, v2):
    inv1 = (g1 / np.sqrt(v1 + EPS)).astype(np.float64)
    inv2 = (g2 / np.sqrt(v2 + EPS)).astype(np.float64)
    wc1 = w1.astype(np.float64) * inv1[:, None, None, None]
    wc2 = w2.astype(np.float64) * inv2[:, None, None, None]
    b1p = (be1.astype(np.float64) + (b1.astype(np.float64) - m1) * inv1)
    b2p = (be2.astype(np.float64) + (b2.astype(np.float64) - m2) * inv2)

    # the device computes feat2 = 2*feat and out2 = 2*out (host halves it):
    # fold 1/2 into conv1 weights, 2x into conv2 weights and bias2.
    wc1 = wc1 * 0.5
    wc2 = wc2 * 2.0
    b2p = b2p * 2.0

    wts = np.zeros((128, 12, 128), np.float32)
    for conv, wc in ((0, wc1), (1, wc2)):
        base = 6 * conv
        for kx in range(3):
            # A taps: dense across row parities, ky = 1 + rp - orp
            for rp in range(2):
                for orp in range(2):
                    ky = 1 + rp - orp
                    wts[64 * rp : 64 * rp + 64, base + kx,
                        64 * orp : 64 * orp + 64] = wc[:, :, ky, kx].T
            # merged BC taps: [0:64]=prev-pair par1 -> out par0 (ky=0);
            # [64:128]=next-pair par0 -> out par1 (ky=2)
            wts[0:64, base + 3 + kx, 0:64] = wc[:, :, 0, kx].T
            wts[64:128, base + 3 + kx, 64:128] = wc[:, :, 2, kx].T
    bias1 = np.tile(b1p.astype(np.float32), 2).reshape(128, 1)
    bias2 = np.tile(b2p.astype(np.float32), 2).reshape(128, 1)

    # u, v from batch 0 (f64 on host, shipped bf16)
    x064 = x0.astype(np.float64)
    a = x064[:, 0::2, 0::2]
    b = x064[:, 0::2, 1::2]
    c = x064[:, 1::2, 0::2]
    d = x064[:, 1::2, 1::2]
    e = a - d
    f = b - c
    g = 1.0 / (1.0 + (e * e + f * f) / 2.0)
    # u2 = 2u = g*e, v2 = 2v = g*f (feat2 = 2*feat convention)
    uv = np.concatenate(
        [(g * e).astype(np.float32), (g * f).astype(np.float32)], axis=0
    ).astype(BFNP)
    return wts, bias1, bias2, np.ascontiguousarray(uv)


def kernel(x, w1, b1, g1, be1, m1, v1, w2, b2, g2, be2, m2, v2, **_kw):
    x = np.asarray(x, dtype=np.float32)
    wts, bias1, bias2, uv = _host_prep(
        x[0], np.asarray(w1), np.asarray(b1), np.asarray(g1), np.asarray(be1),
        np.asarray(m1), np.asarray(v1), np.asarray(w2), np.asarray(b2),
        np.asarray(g2), np.asarray(be2), np.asarray(m2), np.asarray(v2),
    )
    xb = np.ascontiguousarray(x.astype(BFNP))
    in_maps = []
    for c in range(N_CORES):
        in_maps.append(
            {
                "xs": np.ascontiguousarray(xb[BPC * c : BPC * (c + 1)]),
                "uv": uv,
                "wts": wts,
                "bias1": bias1,
                "bias2": bias2,
            }
        )
    nc = _get_nc()
    try:
        res = run_bass_kernel_spmd(nc, in_maps, list(range(N_CORES)))
    except Exception:
        import time as _time

        _time.sleep(5)
        res = run_bass_kernel_spmd(nc, in_maps, list(range(N_CORES)))
    out = np.concatenate([r["out"] for r in res.results], axis=0)
    return out.astype(np.float32) * 0.5


# revision 31
# speedup vs baseline: 1.1857x; 1.1857x over previous
"""DWT-based Perona-Malik diffusion block on 8 Trainium2 NeuronCores.

Math (see reference): one level of orthonormal Haar DWT, PM diffusion of the
detail subbands computed from batch 0 only and broadcast to every batch, IDWT,
then conv3x3 -> BN -> relu -> conv3x3 -> BN -> +feat.

Algebraic reductions:
  g = 1 / (1 + (e^2 + f^2)/2) with e = a - d, f = b - c of batch 0's 2x2
  blocks; u = g*e/2, v = g*f/2 (computed on host, shipped as bf16).
  Per batch, with s1 = a + d, s2 = b + c:
    feat[2i,2j]   = s1/2 + u     feat[2i,2j+1] = s2/2 + v
    feat[2i+1,2j] = s2/2 - v     feat[2i+1,2j+1] = s1/2 - u
  BN folds into the conv weights/biases (host).

Device layout: partition = ch + 64*row_parity (loaded that way straight from
DRAM); columns stored parity-split ([2, 130] sections with guard cols) so all
DVE writes are packed bf16 (4x mode). The 3x3 convs run as bf16 matmuls over
stacked (channel x row-parity) partitions: per output 2-pair psum bank, 6
accumulation passes -- 3 "A" taps (same row-pair, dense 128x128 weights) from
the natural tile, 3 "BC" taps (adjacent row pairs) from a partition-swapped,
slot-shifted companion tile (frBC/ztBC) with block-diagonal weights.

Sharding: pure data parallelism, 2 batches per core; u,v replicated.
"""

import sys

for _p in ("/opt/pypackages", "/opt/trn_rl_repo"):
    if _p not in sys.path:
        sys.path.insert(0, _p)

import numpy as np
import ml_dtypes

import concourse.bass as bass
import concourse.mybir as mybir
import concourse.tile as tile_mod
from concourse.bass_utils import run_bass_kernel_spmd
from concourse.tile import TileContext
from concourse.vector_clock import ScopedClock

F32 = mybir.dt.float32
BF16 = mybir.dt.bfloat16
AF = mybir.ActivationFunctionType
ALU = mybir.AluOpType
BFNP = ml_dtypes.bfloat16

N_CORES = 8
B, C, H, W = 16, 64, 256, 256
BPC = B // N_CORES  # batches per core
NPAIR = H // 2  # 128 row pairs per image
G = 8  # row pairs per chunk
NCHUNK = NPAIR // G  # 16
NQ = W // 2  # column pairs
SEC = NQ + 2  # per-parity-section cols incl guard cols 0 and NQ+1
EPS = 1e-5


def _patched_drain_and_barrier(self, tick_clock, wait_clock):
    # This walrus build rejects >1 sync-wait command per instruction; put the
    # tile-exit drain's waits on individual nops instead.
    nc = self.nc
    collector = nc.sync.nop(nofuse=True)
    wait_clock.add_sem_waits(
        collector.ins, ScopedClock({None: tick_clock.global_clock})
    )
    si = collector.ins.sync_info
    waits = list(si.on_wait) if si is not None else []
    if si is not None:
        si.on_wait = waits[:1]
    for w in waits[1:]:
        n = nc.sync.nop(nofuse=True)
        n.ins.sync_info = mybir.SyncInfo(on_wait=[w], on_update=[])
    nc.sync.drain()
    nc.all_engine_barrier()
    popped = nc._tile_sem_poison_stack.pop()
    assert popped is self._sem_poison
    nc.clear_and_free_semaphores(list(self.sems.allocated().values()))
    nc.all_engine_barrier()


tile_mod.TileContext._drain_and_barrier = _patched_drain_and_barrier

# NOTE: unlike the f32r predecessor, this kernel's bf16 weights legalize into
# standalone InstLdweights, which walrus rejects under --enable-ldw-opt=true;
# the default (=false) is required, so no run_command patching here.

F32R = mybir.dt.float32r


def split_multi_waits(nc):
    """Move extra sync-waits onto preceding single-wait nops (same engine)."""
    for fn in nc.m.functions:
        for blk in fn.blocks:
            new_insts = []
            for inst in blk.instructions:
                si = inst.sync_info
                waits = list(si.on_wait) if si is not None else []
                if len(waits) > 1:
                    for w in waits[:-1]:
                        n = mybir.InstNoOp(
                            name=nc.get_next_instruction_name(), ins=[], outs=[]
                        )
                        n.engine = inst.engine
                        n.bass_nofuse = True
                        n.sync_info = mybir.SyncInfo(on_wait=[w], on_update=[])
                        new_insts.append(n)
                    si.on_wait = waits[-1:]
                new_insts.append(inst)
            blk.instructions = new_insts


# Tap tables: per output column parity, the 6 accumulation passes of a conv.
# Entries: (w_col_offset, src_kind, src_parity, col_shift); w cols 0..2 are
# the dense same-pair "A" taps (kx 0..2), 3..5 the merged adjacent-pair "BC"
# taps. src_kind 0 = natural tile (fr/zt), 1 = companion tile (frBC/ztBC).
TAPS = {
    0: [(0, 0, 1, -1), (1, 0, 0, 0), (2, 0, 1, 0),
        (3, 1, 1, -1), (4, 1, 0, 0), (5, 1, 1, 0)],
    1: [(0, 0, 0, 0), (1, 0, 1, 0), (2, 0, 0, 1),
        (3, 1, 0, 0), (4, 1, 1, 0), (5, 1, 0, 1)],
}


def _build_nc():
    nc = bass.Bass("TRN2", target_bir_lowering=False, debug=False,
                   num_devices=N_CORES)

    xs_d = nc.dram_tensor("xs", [BPC, C, H, W], BF16, kind="ExternalInput").ap()
    uv_d = nc.dram_tensor("uv", [128, NPAIR, NQ], BF16,
                          kind="ExternalInput").ap()
    wts_d = nc.dram_tensor("wts", [128, 12, 128], F32R,
                           kind="ExternalInput").ap()
    b1_d = nc.dram_tensor("bias1", [128, 1], F32, kind="ExternalInput").ap()
    b2_d = nc.dram_tensor("bias2", [128, 1], F32, kind="ExternalInput").ap()
    out_d = nc.dram_tensor("out", [BPC, C, H, W], BF16,
                           kind="ExternalOutput").ap()

    NIT = BPC * NCHUNK  # 32 flat chunk iterations

    with TileContext(nc) as tc, nc.allow_low_precision(
        reason="2e-2 rel-err budget tolerates bf16 activations and weights"
    ):
        with (
            tc.tile_pool(name="const", bufs=1) as cpool,
            tc.tile_pool(name="xin", bufs=2) as xpool,
            tc.tile_pool(name="s12", bufs=2) as spool,
            tc.tile_pool(name="fr", bufs=3) as frpool,
            tc.tile_pool(name="frbc", bufs=2) as fbpool,
            tc.tile_pool(name="zt", bufs=3) as zpool,
            tc.tile_pool(name="ztbc", bufs=3) as zbpool,
            tc.tile_pool(name="outb", bufs=2) as opool,
            tc.tile_pool(name="psum1", bufs=2, space="PSUM") as p1pool,
            tc.tile_pool(name="psum2", bufs=2, space="PSUM") as p2pool,
        ):
            wts_sb = cpool.tile([128, 12, 128], F32R)
            b1_sb = cpool.tile([128, 1], F32)
            b2_sb = cpool.tile([128, 1], F32)
            uv_sb = cpool.tile([128, NPAIR, NQ], BF16)

            def load_consts():
                # first x chunk is already in flight; uv streams in pieces
                # behind the early x loads so prep never waits on the bulk
                nc.sync.dma_start(
                    out=uv_sb[:, 0 : 2 * G, :], in_=uv_d[:, 0 : 2 * G, :]
                )
                nc.sync.dma_start(out=wts_sb[:], in_=wts_d[:])
                nc.sync.dma_start(out=b1_sb[:], in_=b1_d[:])
                nc.sync.dma_start(out=b2_sb[:], in_=b2_d[:])

            def load_uv_piece(j):
                # pieces of 28 pairs at iterations -1..2 (16 + 4*28 = 128)
                lo = 2 * G + 28 * j
                hi = min(lo + 28, NPAIR)
                if lo < hi:
                    nc.sync.dma_start(
                        out=uv_sb[:, lo:hi, :], in_=uv_d[:, lo:hi, :]
                    )

            x_tiles = {}
            fr_tiles = {}
            frbc_tiles = {}
            z_tiles = {}
            zbc_tiles = {}

            def load(i):
                bi, k = divmod(i, NCHUNK)
                # partition = channel only; row parity lives in the free dim
                # so the cross-parity s1/s2 adds have equal input bases
                xc = xpool.tile([64, G, 2, W], BF16)
                x_tiles[i] = xc
                rows = xs_d[bi, :, 2 * G * k : 2 * G * (k + 1), :].rearrange(
                    "c (j r) w -> c j r w", r=2
                )
                if i == 0:
                    # pipeline fill: first 5 pairs arrive (and flow) first
                    nc.sync.dma_start(out=xc[:, 0:5], in_=rows[:, 0:5])
                    nc.sync.dma_start(out=xc[:, 5:G], in_=rows[:, 5:G])
                else:
                    nc.sync.dma_start(out=xc[:], in_=rows)

            def prep(i):
                bi, k = divmod(i, NCHUNK)
                xc = x_tiles.pop(i)
                s12 = spool.tile([128, G, NQ], BF16)
                fr = frpool.tile([128, G, 2, SEC], F32R)
                fr_tiles[i] = fr
                body = slice(1, 1 + NQ)
                subs = (slice(0, 5), slice(5, G)) if i == 0 else (slice(0, G),)
                for sl in subs:
                    pr = slice(G * k + sl.start, G * k + sl.stop)
                    u_ap = uv_sb[0:64, pr, :]
                    v_ap = uv_sb[64:128, pr, :]
                    nc.vector.tensor_add(
                        out=s12[0:64, sl], in0=xc[:, sl, 0, 0::2],
                        in1=xc[:, sl, 1, 1::2],
                    )
                    nc.vector.tensor_add(
                        out=s12[64:128, sl], in0=xc[:, sl, 0, 1::2],
                        in1=xc[:, sl, 1, 0::2],
                    )
                    # feat2 = 2*feat = s +/- u2 (the 1/2 is folded into w1 and
                    # the host-side output halving)
                    nc.vector.tensor_add(
                        out=fr[0:64, sl, 0, body], in0=s12[0:64, sl], in1=u_ap
                    )
                    nc.vector.tensor_add(
                        out=fr[0:64, sl, 1, body], in0=s12[64:128, sl],
                        in1=v_ap,
                    )
                    nc.vector.tensor_sub(
                        out=fr[64:128, sl, 0, body], in0=s12[64:128, sl],
                        in1=v_ap,
                    )
                    nc.vector.tensor_sub(
                        out=fr[64:128, sl, 1, body], in0=s12[0:64, sl],
                        in1=u_ap,
                    )
                nc.vector.memset(fr[:, :, :, 0:1].bitcast(F32), 0.0)
                nc.vector.memset(fr[:, :, :, SEC - 1 : SEC].bitcast(F32), 0.0)

            def build_bc_early(i):
                """frBC main body: needs only fr(i) and fr(i-1) -- emitted
                right after prep(i) so the copies land well before conv1(i)."""
                bi, k = divmod(i, NCHUNK)
                dst = fbpool.tile([128, G, 2, SEC], F32R)
                frbc_tiles[i] = dst
                src = fr_tiles[i]
                if i == 0:
                    nc.gpsimd.dma_start(out=dst[0:64, 1:5], in_=src[64:128, 0:4])
                    nc.gpsimd.dma_start(out=dst[64:128, 0:4], in_=src[0:64, 1:5])
                    nc.gpsimd.dma_start(
                        out=dst[0:64, 5:G], in_=src[64:128, 4 : G - 1]
                    )
                    nc.gpsimd.dma_start(
                        out=dst[64:128, 4 : G - 1], in_=src[0:64, 5:G]
                    )
                else:
                    nc.gpsimd.dma_start(
                        out=dst[0:64, 1:G], in_=src[64:128, 0 : G - 1]
                    )
                    nc.gpsimd.dma_start(
                        out=dst[64:128, 0 : G - 1], in_=src[0:64, 1:G]
                    )
                if k == 0:
                    nc.vector.memset(dst[0:64, 0:1].bitcast(F32), 0.0)
                else:
                    nc.gpsimd.dma_start(
                        out=dst[0:64, 0:1],
                        in_=fr_tiles[i - 1][64:128, G - 1 : G],
                    )

            def build_bc_late(i):
                """frBC forward halo: needs fr(i+1)."""
                bi, k = divmod(i, NCHUNK)
                dst = frbc_tiles[i]
                if k == NCHUNK - 1:
                    nc.vector.memset(dst[64:128, G - 1 : G].bitcast(F32), 0.0)
                else:
                    nc.gpsimd.dma_start(
                        out=dst[64:128, G - 1 : G],
                        in_=fr_tiles[i + 1][0:64, 0:1],
                    )

            def emit_conv(psum, wbase, half, nat, comp):
                """12 matmuls (512 rows each): one half-chunk (4 pairs) of one
                conv. Each output col-parity accumulates in its own psum bank;
                weight-major so each w col is loaded once, used twice."""
                ts = slice(4 * half, 4 * half + 4)
                for wcol in range(6):
                    w_ap = wts_sb[:, wbase + wcol, :]
                    for p_out in (0, 1):
                        _, kind, p_in, shift = TAPS[p_out][wcol]
                        src = (nat if kind == 0 else comp)[
                            :, ts, p_in, 1 + shift : 1 + shift + NQ
                        ]
                        nc.tensor.matmul(
                            psum[:, p_out, :, :],
                            w_ap,
                            src,
                            start=(wcol == 0),
                            stop=(wcol == 5),
                        )

            def zbc_alloc(j):
                bi, k = divmod(j, NCHUNK)
                zbc = zbpool.tile([128, G, 2, SEC], F32R)
                zbc_tiles[j] = zbc
                nc.vector.memset(zbc[:, :, :, 0:1].bitcast(F32), 0.0)
                nc.vector.memset(zbc[:, :, :, SEC - 1 : SEC].bitcast(F32), 0.0)
                if k == 0:
                    nc.vector.memset(zbc[0:64, 0:1].bitcast(F32), 0.0)
                if k == NCHUNK - 1:
                    nc.vector.memset(zbc[64:128, G - 1 : G].bitcast(F32), 0.0)

            def conv1(i, halves):
                bi, k = divmod(i, NCHUNK)
                fr = fr_tiles[i]
                fbc = frbc_tiles[i]
                if 0 in halves:
                    zt = zpool.tile([128, G, 2, SEC], F32R)
                    z_tiles[i] = zt
                else:
                    zt = z_tiles[i]
                zbc = zbc_tiles[i]
                body = slice(1, 1 + NQ)

                def act(out_ap, in_ap, bias):
                    nc.scalar.activation(
                        out_ap, in_ap, AF.Relu, bias=bias, scale=1.0
                    )

                bl = b1_sb[0:64, 0:1]
                bh = b1_sb[64:128, 0:1]
                for h in halves:
                    ps = p1pool.tile([128, 2, 4, NQ], F32)
                    emit_conv(ps, 0, h, fr, fbc)
                    for p in (0, 1):
                        # natural tile: both parities
                        act(zt[:, 4 * h : 4 * h + 4, p, body],
                            ps[:, p, :, :], b1_sb[:, 0:1])
                        # companion: par1 -> zbc[0:64] at slot pair+1,
                        # par0 -> zbc[64:128] at slot pair-1
                        if h == 0:
                            act(zbc[0:64, 1:5, p, body],
                                ps[64:128, p, 0:4, :], bh)
                            if k > 0:
                                act(zbc_tiles[i - 1][64:128, G - 1 : G, p, body],
                                    ps[0:64, p, 0:1, :], bl)
                            act(zbc[64:128, 0:3, p, body],
                                ps[0:64, p, 1:4, :], bl)
                        else:
                            act(zbc[0:64, 5:8, p, body],
                                ps[64:128, p, 0:3, :], bh)
                            if k < NCHUNK - 1:
                                act(zbc_tiles[i + 1][0:64, 0:1, p, body],
                                    ps[64:128, p, 3:4, :], bh)
                            act(zbc[64:128, 3:7, p, body],
                                ps[0:64, p, 0:4, :], bl)
                if 0 in halves:
                    nc.vector.memset(zt[:, :, :, 0:1].bitcast(F32), 0.0)
                    nc.vector.memset(
                        zt[:, :, :, SEC - 1 : SEC].bitcast(F32), 0.0
                    )

            def conv2(i):
                bi, k = divmod(i, NCHUNK)
                zt = z_tiles[i]
                zbc = zbc_tiles[i]
                fr = fr_tiles[i]
                ot = opool.tile([128, G, W], BF16)
                for h in range(2):
                    ps = p2pool.tile([128, 2, 4, NQ], F32)
                    emit_conv(ps, 6, h, zt, zbc)
                    for p in (0, 1):
                        nc.vector.scalar_tensor_tensor(
                            out=ot[:, 4 * h : 4 * h + 4, p::2],
                            in0=ps[:, p, :, :],
                            scalar=b2_sb[:, 0:1],
                            in1=fr[:, 4 * h : 4 * h + 4, p, 1 : 1 + NQ].bitcast(F32),
                            op0=ALU.add, op1=ALU.add,
                        )
                orows = out_d[bi, :, 2 * G * k : 2 * G * (k + 1), :].rearrange(
                    "c (j r) w -> c j r w", r=2
                )
                if i == NIT - 1:
                    # tail latency: ship each 4-pair half as soon as ready
                    for hh in range(2):
                        for r in range(2):
                            nc.sync.dma_start(
                                out=orows[:, 4 * hh : 4 * hh + 4, r, :],
                                in_=ot[64 * r : 64 * (r + 1),
                                       4 * hh : 4 * hh + 4],
                            )
                else:
                    for r in range(2):
                        nc.sync.dma_start(
                            out=orows[:, :, r, :], in_=ot[64 * r : 64 * (r + 1)]
                        )


            for i in range(-1, NIT + 1):
                if 0 <= i + 1 < NIT:
                    load(i + 1)
                    if i == -1:
                        load_consts()
                    if -1 <= i <= 2:
                        load_uv_piece(i + 1)
                    zbc_alloc(i + 1)
                    prep(i + 1)
                if 0 <= i < NIT:
                    build_bc_early(i)
                    build_bc_late(i)
                    conv1(i, (0, 1))
                if 0 <= i - 1 < NIT:
                    conv2(i - 1)

    split_multi_waits(nc)
    return nc


_NC_CACHE = {}


def _get_nc():
    if "nc" not in _NC_CACHE:
        _NC_CACHE["nc"] = _build_nc()
    return _NC_CACHE["nc"]


def _host_prep(x0, w1, b1, g1, be1, m1, v1, w2, b2, g2, be2, m2, v2):
    inv1 = (g1 / np.sqrt(v1 + EPS)).astype(np.float64)
    inv2 = (g2 / np.sqrt(v2 + EPS)).astype(np.float64)
    wc1 = w1.astype(np.float64) * inv1[:, None, None, None]
    wc2 = w2.astype(np.float64) * inv2[:, None, None, None]
    b1p = (be1.astype(np.float64) + (b1.astype(np.float64) - m1) * inv1)
    b2p = (be2.astype(np.float64) + (b2.astype(np.float64) - m2) * inv2)

    # the device computes feat2 = 2*feat and out2 = 2*out (host halves it):
    # fold 1/2 into conv1 weights, 2x into conv2 weights and bias2.
    wc1 = wc1 * 0.5
    wc2 = wc2 * 2.0
    b2p = b2p * 2.0

    wts = np.zeros((128, 12, 128), np.float32)
    for conv, wc in ((0, wc1), (1, wc2)):
        base = 6 * conv
        for kx in range(3):
            # A taps: dense across row parities, ky = 1 + rp - orp
            for rp in range(2):
                for orp in range(2):
                    ky = 1 + rp - orp
                    wts[64 * rp : 64 * rp + 64, base + kx,
                        64 * orp : 64 * orp + 64] = wc[:, :, ky, kx].T
            # merged BC taps: [0:64]=prev-pair par1 -> out par0 (ky=0);
            # [64:128]=next-pair par0 -> out par1 (ky=2)
            wts[0:64, base + 3 + kx, 0:64] = wc[:, :, 0, kx].T
            wts[64:128, base + 3 + kx, 64:128] = wc[:, :, 2, kx].T
    bias1 = np.tile(b1p.astype(np.float32), 2).reshape(128, 1)
    bias2 = np.tile(b2p.astype(np.float32), 2).reshape(128, 1)

    # u, v from batch 0 (f64 on host, shipped bf16)
    x064 = x0.astype(np.float64)
    a = x064[:, 0::2, 0::2]
    b = x064[:, 0::2, 1::2]
    c = x064[:, 1::2, 0::2]
    d = x064[:, 1::2, 1::2]
    e = a - d
    f = b - c
    g = 1.0 / (1.0 + (e * e + f * f) / 2.0)
    # u2 = 2u = g*e, v2 = 2v = g*f (feat2 = 2*feat convention)
    uv = np.concatenate(
        [(g * e).astype(np.float32), (g * f).astype(np.float32)], axis=0
    ).astype(BFNP)
    return wts, bias1, bias2, np.ascontiguousarray(uv)


def kernel(x, w1, b1, g1, be1, m1, v1, w2, b2, g2, be2, m2, v2, **_kw):
    x = np.asarray(x, dtype=np.float32)
    wts, bias1, bias2, uv = _host_prep(
        x[0], np.asarray(w1), np.asarray(b1), np.asarray(g1), np.asarray(be1),
        np.asarray(m1), np.asarray(v1), np.asarray(w2), np.asarray(b2),
        np.asarray(g2), np.asarray(be2), np.asarray(m2), np.asarray(v2),
    )
    xb = np.ascontiguousarray(x.astype(BFNP))
    in_maps = []
    for c in range(N_CORES):
        in_maps.append(
            {
                "xs": np.ascontiguousarray(xb[BPC * c : BPC * (c + 1)]),
                "uv": uv,
                "wts": wts,
                "bias1": bias1,
                "bias2": bias2,
            }
        )
    nc = _get_nc()
    try:
        res = run_bass_kernel_spmd(nc, in_maps, list(range(N_CORES)))
    except Exception:
        import time as _time

        _time.sleep(5)
        res = run_bass_kernel_spmd(nc, in_maps, list(range(N_CORES)))
    out = np.concatenate([r["out"] for r in res.results], axis=0)
    return out.astype(np.float32) * 0.5
